# revision 20
# baseline (speedup 1.0000x reference)
"""DescriptorLoss kernel for Trainium2 (8 NeuronCores, SPMD data-parallel).

Math:
    d[b,ij,kl] = sum_c desc0[b,c,ij] * desc1[b,c,kl]
    loss = mean(where(mask, 250*relu(1 - d), relu(d - 0.2)))

Per core: shard = (batch, i-slab) -> 1024 ij rows x 4096 kl cols, processed
as 16 half-groups (hg) of 128 rows x 2048 cols (one 4-bank PSUM tile each).
The PE computes d' = 5*d with fp8e4m3 matmuls (same PE rate as bf16, less
DMA/SBUF; quantization error ~2e-4 << tolerance) and injects the mask with
one extra matmul per 512 cols:  u = d' - 2048*m  (diag(-1) @ m8, fp8e5m2),
putting both hinges in disjoint scalar ranges (|d'| < ~1100 << 2048).

Reductions on TRN2 run at 1 elem/lane/cycle on ACT/DVE only (no DVE
perf-mode uops for accumulating ops; GPSIMD cannot touch PSUM), and two
engines reading the same PSUM banks contend (+20%) or serialize. So each
PSUM tile is read EXACTLY ONCE, by ACT, with an information-preserving
Leaky-ReLU that simultaneously evaluates the positive hinge:

  ACT pass1 (PSUM): out1 = |u - 1| fp16->SBUF, acc1 = sum|u-1|
      A = sum relu(u-1) = (Su - N)/2 + acc1/2   via relu(x) = (x+|x|)/2;
      Su = sum(u) computed EXACTLY on the host from the quantized inputs
      (a5q . colsum(bmq) - 2048*popcount, a few MFLOP).
  DVE pass2 (SBUF, decoupled from PSUM): acc2 = sum max(out1, 2044)
      B = sum relu(-u-2043) = acc2 - 2044*N   (masked: |u-1| = 1-u so
      |u-1|-2044 = -u-2043; unmasked: |u-1| < 1200 < 2044 clamps away).

Host: loss = sum(A + 250*B)/5 / count.  PSUM hold is a single 1.97us pass,
so the 2-buffer PSUM rotation sustains ACT at ~full rate while DVE trails
one stage behind on SBUF data; PE (128 small matmuls) fills the gaps and
stays un-throttled.
"""

import numpy as np
import ml_dtypes

import concourse.bacc as bacc
import concourse.mybir as mybir
import concourse.tile as tile
from concourse.bass_utils import run_bass_kernel_spmd

B, D, H, W = 2, 128, 64, 64
N_CORES = 8
IJ = H * W                 # 4096
ROWS_PER_CORE = IJ // 4    # 1024
G = ROWS_PER_CORE // 128   # 8 row groups of 128
HG_COLS = 2048             # half-group column width (one 4-bank PSUM tile)
N_HG = G * 2               # 16 half-groups per core
C = 2048.0                 # mask inject magnitude
THR2 = C - 4.0             # 2044: pass2 clamp constant (fp16-exact)

MMF = 512                  # matmul moving free dim (cols per matmul)

_cached = {}


def _build_program():
    nc = bacc.Bacc("TRN2")
    f32 = mybir.dt.float32
    f16 = mybir.dt.float16
    f8e4 = mybir.dt.float8e4
    f8e5 = mybir.dt.float8e5
    Alu = mybir.AluOpType
    Act = mybir.ActivationFunctionType

    a5 = nc.declare_dram_parameter("a5", [D, G, 128], f8e4, isOutput=False)
    bm = nc.declare_dram_parameter("bm", [D, IJ], f8e4, isOutput=False)
    idn = nc.declare_dram_parameter("idn", [D, D], f8e5, isOutput=False)
    m8 = nc.declare_dram_parameter("m8", [128, N_HG, HG_COLS], f8e5, isOutput=False)
    accs_out = nc.declare_dram_parameter("accs", [128, 2 * N_HG], f32, isOutput=True)

    with tile.TileContext(nc) as tc:
        with (
            tc.tile_pool(name="desc", bufs=1) as desc_pool,
            tc.tile_pool(name="mask", bufs=4) as mask_pool,
            tc.tile_pool(name="out1", bufs=4) as o1_pool,
            tc.tile_pool(name="gout", bufs=2) as g_pool,
            tc.tile_pool(name="accs", bufs=1) as acc_pool,
            tc.tile_pool(name="psd", bufs=2, space="PSUM") as psum_pool,
        ):
            a5_t = desc_pool.tile([D, G, 128], f8e4, tag="a5")
            bm_t = desc_pool.tile([D, IJ], f8e4, tag="bm")
            id_t = desc_pool.tile([D, D], f8e5, tag="idn")
            bias_a = desc_pool.tile([128, 1], f32, tag="ba")
            bias_b = desc_pool.tile([128, 1], f32, tag="bb")
            # split the big input DMAs so the first matmuls can start early:
            # the first half-group needs only a5[g0], the first 2048 bm
            # columns, the identity and its own mask
            nc.sync.dma_start(a5_t[:, 0, :], a5[:, 0, :])
            nc.sync.dma_start(bm_t[:, :HG_COLS], bm[:, :HG_COLS])
            nc.sync.dma_start(id_t[:], idn[:])
            nc.gpsimd.memset(bias_a[:], -1.0)
            nc.gpsimd.memset(bias_b[:], -THR2)

            accA_t = acc_pool.tile([128, N_HG], f32, tag="accsA")
            accB_t = acc_pool.tile([128, N_HG], f32, tag="accsB")

            for hg in range(N_HG):
                g, h = hg // 2, hg % 2
                c0 = h * HG_COLS

                mm_t = mask_pool.tile([128, HG_COLS], f8e5, tag="m8")
                nc.sync.dma_start(mm_t[:], m8[:, hg, :])
                if hg == 0:
                    # bulk loads issued after the first mask so hg0's inject
                    # chain is not stuck behind them in the DMA issue queue
                    nc.sync.dma_start(bm_t[:, HG_COLS:], bm[:, HG_COLS:])
                    nc.sync.dma_start(a5_t[:, 1:, :], a5[:, 1:, :])

                psum_t = psum_pool.tile([128, HG_COLS], f32, tag="d")
                for j in range(HG_COLS // MMF):
                    js = slice(j * MMF, (j + 1) * MMF)
                    cs = slice(c0 + j * MMF, c0 + (j + 1) * MMF)
                    nc.tensor.matmul(
                        psum_t[:, js], a5_t[:, g, :], bm_t[:, cs],
                        start=True, stop=False,
                    )
                for j in range(HG_COLS // MMF):
                    js = slice(j * MMF, (j + 1) * MMF)
                    nc.tensor.matmul(
                        psum_t[:, js], id_t[:], mm_t[:, js],
                        start=False, stop=True,
                    )

                out1 = o1_pool.tile([128, HG_COLS], f16, tag="o1")
                nc.scalar.activation(
                    out1[:], psum_t[:], Act.Abs,
                    bias=bias_a[:], scale=1.0,
                    accum_out=accA_t[:, hg:hg + 1],
                )
                g2 = g_pool.tile([128, HG_COLS], f16, tag="g")
                if hg == 0:
                    nc.scalar.activation(
                        g2[:], out1[:], Act.Relu,
                        bias=bias_b[:], scale=1.0,
                        accum_out=accB_t[:, hg:hg + 1],
                    )
                else:
                    nc.vector.tensor_scalar(
                        g2[:], out1[:], THR2, None,
                        op0=Alu.max, op1=Alu.add,
                        accum_out=accB_t[:, hg:hg + 1],
                    )

            nc.sync.dma_start(accs_out[:, :N_HG], accA_t[:])
            nc.sync.dma_start(accs_out[:, N_HG:], accB_t[:])

    nc.finalize()
    return nc


def _prep_inputs(descriptors_0, descriptors_1, similarity_mask):
    d0 = np.asarray(descriptors_0, dtype=np.float32)
    d1 = np.asarray(descriptors_1, dtype=np.float32)
    mkv = np.asarray(similarity_mask)
    idn = np.zeros((D, D), dtype=np.float32)
    np.fill_diagonal(idn, -1.0)
    idn = np.ascontiguousarray(idn.astype(ml_dtypes.float8_e5m2))
    in_maps = []
    su_list = []
    for c in range(N_CORES):
        b = c >> 2
        r0 = (c & 3) * ROWS_PER_CORE
        a5 = (d0[b].reshape(D, IJ)[:, r0:r0 + ROWS_PER_CORE] * np.float32(5.0))
        a5q8 = a5.astype(ml_dtypes.float8_e4m3fn)
        a5q = a5q8.astype(np.float32)            # [128 chan, 1024 rows]
        bmq8 = d1[b].reshape(D, IJ).astype(ml_dtypes.float8_e4m3fn)
        bmq = bmq8.astype(np.float32)
        mk = mkv[b].reshape(IJ, IJ)[r0:r0 + ROWS_PER_CORE]  # [1024, 4096] bool
        # [row(G,128), col(2,2048)] -> [128, hg=(G,2), 2048]
        m4k = (mk.astype(np.float32) * np.float32(C)).reshape(G, 128, 2, HG_COLS)
        m8v = np.ascontiguousarray(m4k.transpose(1, 0, 2, 3).reshape(
            128, N_HG, HG_COLS
        )).astype(ml_dtypes.float8_e5m2)
        # host-exact Su_total = sum(u) over the whole slab
        bsum = bmq.sum(axis=1, dtype=np.float64)
        su_tot = float((a5q.astype(np.float64).T @ bsum).sum()) - float(C) * float(
            mk.sum(dtype=np.int64)
        )
        su_list.append(su_tot)
        in_maps.append(
            {
                "a5": np.ascontiguousarray(a5q8.reshape(D, G, 128)),
                "bm": np.ascontiguousarray(bmq8),
                "idn": idn,
                "m8": m8v,
            }
        )
    _cached["su"] = su_list
    return in_maps


def _run(in_maps, **kwargs):
    if "nc" not in _cached:
        _cached["nc"] = _build_program()
    return run_bass_kernel_spmd(_cached["nc"], in_maps, list(range(N_CORES)), **kwargs)


def _combine(results):
    su_list = _cached["su"]
    n_core = ROWS_PER_CORE * IJ                  # elements per core
    total = 0.0
    for c, r in enumerate(results):
        accs = r["accs"].astype(np.float64)      # [128, 2*N_HG]
        acc1 = accs[:, :N_HG].sum()
        acc2 = accs[:, N_HG:].sum()
        A = 0.5 * (su_list[c] - n_core) + 0.5 * acc1
        # hg0's pass2 runs on ACT as relu(out1-2044) whose accum is B
        # directly; the other 15 hgs accumulate max(out1, 2044) = B + 2044*N
        Bv = acc2 - THR2 * (n_core - 128 * HG_COLS)
        total += (A + 250.0 * Bv) / 5.0
    return np.float32(total / float(B * IJ * IJ))


def kernel(descriptors_0, descriptors_1, similarity_mask):
    in_maps = _prep_inputs(descriptors_0, descriptors_1, similarity_mask)
    res = _run(in_maps)
    return _combine(res.results)


# revision 21
# speedup vs baseline: 1.0332x; 1.0332x over previous
"""DescriptorLoss kernel for Trainium2 (8 NeuronCores, SPMD data-parallel).

Math:
    d[b,ij,kl] = sum_c desc0[b,c,ij] * desc1[b,c,kl]
    loss = mean(where(mask, 250*relu(1 - d), relu(d - 0.2)))

Per core: shard = (batch, i-slab) -> 1024 ij rows x 4096 kl cols, processed
as 16 half-groups (hg) of 128 rows x 2048 cols (one 4-bank PSUM tile each).
The PE computes d' = 5*d with fp8e4m3 matmuls (same PE rate as bf16, less
DMA/SBUF; quantization error ~2e-4 << tolerance) and injects the mask with
one extra matmul per 512 cols:  u = d' - 2048*m  (diag(-1) @ m8, fp8e5m2),
putting both hinges in disjoint scalar ranges (|d'| < ~1100 << 2048).

Reductions on TRN2 run at 1 elem/lane/cycle on ACT/DVE only (no DVE
perf-mode uops for accumulating ops; GPSIMD cannot touch PSUM), and two
engines reading the same PSUM banks contend (+20%) or serialize. So each
PSUM tile is read EXACTLY ONCE, by ACT, with an information-preserving
Leaky-ReLU that simultaneously evaluates the positive hinge:

  ACT pass1 (PSUM): out1 = |u - 1| fp16->SBUF, acc1 = sum|u-1|
      A = sum relu(u-1) = (Su - N)/2 + acc1/2   via relu(x) = (x+|x|)/2;
      Su = sum(u) computed EXACTLY on the host from the quantized inputs
      (a5q . colsum(bmq) - 2048*popcount, a few MFLOP).
  DVE pass2 (SBUF, decoupled from PSUM): acc2 = sum max(out1, 2044)
      B = sum relu(-u-2043) = acc2 - 2044*N   (masked: |u-1| = 1-u so
      |u-1|-2044 = -u-2043; unmasked: |u-1| < 1200 < 2044 clamps away).

Host: loss = sum(A + 250*B)/5 / count.  PSUM hold is a single 1.97us pass,
so the 2-buffer PSUM rotation sustains ACT at ~full rate while DVE trails
one stage behind on SBUF data; PE (128 small matmuls) fills the gaps and
stays un-throttled.
"""

import numpy as np
import ml_dtypes

import concourse.bacc as bacc
import concourse.mybir as mybir
import concourse.tile as tile
from concourse.bass_utils import run_bass_kernel_spmd

B, D, H, W = 2, 128, 64, 64
N_CORES = 8
IJ = H * W                 # 4096
ROWS_PER_CORE = IJ // 4    # 1024
G = ROWS_PER_CORE // 128   # 8 row groups of 128
HG_COLS = 2048             # half-group column width (one 4-bank PSUM tile)
N_HG = G * 2               # 16 half-groups per core
C = 2048.0                 # mask inject magnitude
THR2 = C - 4.0             # 2044: pass2 clamp constant (fp16-exact)

MMF = 512                  # matmul moving free dim (cols per matmul)

_cached = {}


def _build_program():
    nc = bacc.Bacc("TRN2")
    f32 = mybir.dt.float32
    f16 = mybir.dt.float16
    f8e4 = mybir.dt.float8e4
    f8e5 = mybir.dt.float8e5
    Alu = mybir.AluOpType
    Act = mybir.ActivationFunctionType

    a5 = nc.declare_dram_parameter("a5", [D, G, 128], f8e4, isOutput=False)
    bm = nc.declare_dram_parameter("bm", [D, IJ], f8e4, isOutput=False)
    idn = nc.declare_dram_parameter("idn", [D, D], f8e5, isOutput=False)
    m8 = nc.declare_dram_parameter("m8", [128, N_HG, HG_COLS], f8e5, isOutput=False)
    accs_out = nc.declare_dram_parameter("accs", [128, 2 * N_HG], f32, isOutput=True)

    with tile.TileContext(nc) as tc:
        with (
            tc.tile_pool(name="desc", bufs=1) as desc_pool,
            tc.tile_pool(name="mask", bufs=4) as mask_pool,
            tc.tile_pool(name="out1", bufs=4) as o1_pool,
            tc.tile_pool(name="gout", bufs=2) as g_pool,
            tc.tile_pool(name="accs", bufs=1) as acc_pool,
            tc.tile_pool(name="psd", bufs=2, space="PSUM") as psum_pool,
        ):
            a5_t = desc_pool.tile([D, G, 128], f8e4, tag="a5")
            bm_t = desc_pool.tile([D, IJ], f8e4, tag="bm")
            id_t = desc_pool.tile([D, D], f8e5, tag="idn")
            bias_a = desc_pool.tile([128, 1], f32, tag="ba")
            bias_b = desc_pool.tile([128, 1], f32, tag="bb")
            # split the big input DMAs so the first matmuls can start early:
            # the first half-group needs only a5[g0], the first 2048 bm
            # columns, the identity and its own mask
            nc.sync.dma_start(a5_t[:, 0, :], a5[:, 0, :])
            nc.sync.dma_start(bm_t[:, :HG_COLS], bm[:, :HG_COLS])
            nc.sync.dma_start(id_t[:], idn[:])
            nc.gpsimd.memset(bias_a[:], -1.0)
            nc.gpsimd.memset(bias_b[:], -THR2)

            accA_t = acc_pool.tile([128, N_HG], f32, tag="accsA")
            accB_t = acc_pool.tile([128, N_HG], f32, tag="accsB")

            for hg in range(N_HG):
                g, h = hg // 2, hg % 2
                c0 = h * HG_COLS

                mm_t = mask_pool.tile([128, HG_COLS], f8e5, tag="m8")
                nc.sync.dma_start(mm_t[:], m8[:, hg, :])
                if hg == 0:
                    # bulk loads issued after the first mask so hg0's inject
                    # chain is not stuck behind them in the DMA issue queue
                    nc.sync.dma_start(bm_t[:, HG_COLS:], bm[:, HG_COLS:])
                    nc.sync.dma_start(a5_t[:, 1:, :], a5[:, 1:, :])

                psum_t = psum_pool.tile([128, HG_COLS], f32, tag="d")
                for j in range(HG_COLS // MMF):
                    js = slice(j * MMF, (j + 1) * MMF)
                    cs = slice(c0 + j * MMF, c0 + (j + 1) * MMF)
                    nc.tensor.matmul(
                        psum_t[:, js], a5_t[:, g, :], bm_t[:, cs],
                        start=True, stop=False,
                    )
                for j in range(HG_COLS // MMF):
                    js = slice(j * MMF, (j + 1) * MMF)
                    nc.tensor.matmul(
                        psum_t[:, js], id_t[:], mm_t[:, js],
                        start=False, stop=True,
                    )

                out1 = o1_pool.tile([128, HG_COLS], f16, tag="o1")
                nc.scalar.activation(
                    out1[:], psum_t[:], Act.Abs,
                    bias=bias_a[:], scale=1.0,
                    accum_out=accA_t[:, hg:hg + 1],
                )
                g2 = g_pool.tile([128, HG_COLS], f16, tag="g")
                nc.vector.tensor_scalar(
                    g2[:], out1[:], THR2, None,
                    op0=Alu.max, op1=Alu.add,
                    accum_out=accB_t[:, hg:hg + 1],
                )

            nc.sync.dma_start(accs_out[:, :N_HG], accA_t[:])
            nc.sync.dma_start(accs_out[:, N_HG:], accB_t[:])

    nc.finalize()
    return nc


def _prep_inputs(descriptors_0, descriptors_1, similarity_mask):
    d0 = np.asarray(descriptors_0, dtype=np.float32)
    d1 = np.asarray(descriptors_1, dtype=np.float32)
    mkv = np.asarray(similarity_mask)
    idn = np.zeros((D, D), dtype=np.float32)
    np.fill_diagonal(idn, -1.0)
    idn = np.ascontiguousarray(idn.astype(ml_dtypes.float8_e5m2))
    in_maps = []
    su_list = []
    for c in range(N_CORES):
        b = c >> 2
        r0 = (c & 3) * ROWS_PER_CORE
        a5 = (d0[b].reshape(D, IJ)[:, r0:r0 + ROWS_PER_CORE] * np.float32(5.0))
        a5q8 = a5.astype(ml_dtypes.float8_e4m3fn)
        a5q = a5q8.astype(np.float32)            # [128 chan, 1024 rows]
        bmq8 = d1[b].reshape(D, IJ).astype(ml_dtypes.float8_e4m3fn)
        bmq = bmq8.astype(np.float32)
        mk = mkv[b].reshape(IJ, IJ)[r0:r0 + ROWS_PER_CORE]  # [1024, 4096] bool
        # [row(G,128), col(2,2048)] -> [128, hg=(G,2), 2048]
        m4k = (mk.astype(np.float32) * np.float32(C)).reshape(G, 128, 2, HG_COLS)
        m8v = np.ascontiguousarray(m4k.transpose(1, 0, 2, 3).reshape(
            128, N_HG, HG_COLS
        )).astype(ml_dtypes.float8_e5m2)
        # host-exact Su_total = sum(u) over the whole slab
        bsum = bmq.sum(axis=1, dtype=np.float64)
        su_tot = float((a5q.astype(np.float64).T @ bsum).sum()) - float(C) * float(
            mk.sum(dtype=np.int64)
        )
        su_list.append(su_tot)
        in_maps.append(
            {
                "a5": np.ascontiguousarray(a5q8.reshape(D, G, 128)),
                "bm": np.ascontiguousarray(bmq8),
                "idn": idn,
                "m8": m8v,
            }
        )
    _cached["su"] = su_list
    return in_maps


def _run(in_maps, **kwargs):
    if "nc" not in _cached:
        _cached["nc"] = _build_program()
    return run_bass_kernel_spmd(_cached["nc"], in_maps, list(range(N_CORES)), **kwargs)


def _combine(results):
    su_list = _cached["su"]
    n_core = ROWS_PER_CORE * IJ                  # elements per core
    total = 0.0
    for c, r in enumerate(results):
        accs = r["accs"].astype(np.float64)      # [128, 2*N_HG]
        acc1 = accs[:, :N_HG].sum()
        acc2 = accs[:, N_HG:].sum()
        A = 0.5 * (su_list[c] - n_core) + 0.5 * acc1
        Bv = acc2 - THR2 * n_core
        total += (A + 250.0 * Bv) / 5.0
    return np.float32(total / float(B * IJ * IJ))


def kernel(descriptors_0, descriptors_1, similarity_mask):
    in_maps = _prep_inputs(descriptors_0, descriptors_1, similarity_mask)
    res = _run(in_maps)
    return _combine(res.results)


# revision 24
# speedup vs baseline: 1.1045x; 1.0690x over previous
"""DescriptorLoss kernel for Trainium2 (8 NeuronCores, SPMD data-parallel).

Math:
    d[b,ij,kl] = sum_c desc0[b,c,ij] * desc1[b,c,kl]
    loss = mean(where(mask, 250*relu(1 - d), relu(d - 0.2)))

Per core: shard = (batch, i-slab) -> 1024 ij rows x 4096 kl cols, processed
as 16 half-groups (hg) of 128 rows x 2048 cols (one 4-bank PSUM tile each).
The PE computes d' = 5*d with fp8e4m3 matmuls (same PE rate as bf16, less
DMA/SBUF; quantization error ~2e-4 << tolerance) and injects the mask with
one extra matmul per 512 cols:  u = d' - 2048*m  (diag(-1) @ m8, fp8e5m2),
putting both hinges in disjoint scalar ranges (|d'| < ~1100 << 2048).

Reductions on TRN2 run at 1 elem/lane/cycle on ACT/DVE only (no DVE
perf-mode uops for accumulating ops; GPSIMD cannot touch PSUM), and two
engines reading the same PSUM banks contend (+20%) or serialize. So each
PSUM tile is read EXACTLY ONCE, by ACT, with an information-preserving
Leaky-ReLU that simultaneously evaluates the positive hinge:

  ACT pass1 (PSUM): out1 = |u - 1| fp16->SBUF, acc1 = sum|u-1|
      A = sum relu(u-1) = (Su - N)/2 + acc1/2   via relu(x) = (x+|x|)/2;
      Su = sum(u) computed EXACTLY on the host from the quantized inputs
      (a5q . colsum(bmq) - 2048*popcount, a few MFLOP).
  DVE pass2 (SBUF, decoupled from PSUM): acc2 = sum max(out1, 2044)
      B = sum relu(-u-2043) = acc2 - 2044*N   (masked: |u-1| = 1-u so
      |u-1|-2044 = -u-2043; unmasked: |u-1| < 1200 < 2044 clamps away).

Host: loss = sum(A + 250*B)/5 / count.  PSUM hold is a single 1.97us pass,
so the 2-buffer PSUM rotation sustains ACT at ~full rate while DVE trails
one stage behind on SBUF data; PE (128 small matmuls) fills the gaps and
stays un-throttled.
"""

import numpy as np
import ml_dtypes

import concourse.bacc as bacc
import concourse.mybir as mybir
import concourse.tile as tile
from concourse.bass_utils import run_bass_kernel_spmd

B, D, H, W = 2, 128, 64, 64
N_CORES = 8
IJ = H * W                 # 4096
ROWS_PER_CORE = IJ // 4    # 1024
G = ROWS_PER_CORE // 128   # 8 row groups of 128
HG_COLS = 2048             # half-group column width (one 4-bank PSUM tile)
N_HG = G * 2               # 16 half-groups per core
C = 2048.0                 # mask inject magnitude
THR2 = C - 4.0             # 2044: pass2 clamp constant (fp16-exact)

MMF = 512                  # matmul moving free dim (cols per matmul)

_cached = {}


def _build_program():
    nc = bacc.Bacc("TRN2")
    f32 = mybir.dt.float32
    f16 = mybir.dt.float16
    f8e4 = mybir.dt.float8e4
    f8e5 = mybir.dt.float8e5
    Alu = mybir.AluOpType
    Act = mybir.ActivationFunctionType

    a5 = nc.declare_dram_parameter("a5", [D, G, 128], f8e4, isOutput=False)
    bm = nc.declare_dram_parameter("bm", [D, IJ], f8e4, isOutput=False)
    idn = nc.declare_dram_parameter("idn", [D, D], f8e5, isOutput=False)
    m8 = nc.declare_dram_parameter("m8", [128, N_HG, HG_COLS], f8e5, isOutput=False)
    accs_out = nc.declare_dram_parameter("accs", [128, 2 * N_HG], f32, isOutput=True)

    with tile.TileContext(nc) as tc:
        with (
            tc.tile_pool(name="desc", bufs=1) as desc_pool,
            tc.tile_pool(name="mask", bufs=4) as mask_pool,
            tc.tile_pool(name="out1", bufs=4) as o1_pool,
            tc.tile_pool(name="gout", bufs=2) as g_pool,
            tc.tile_pool(name="accs", bufs=1) as acc_pool,
            tc.tile_pool(name="psd", bufs=2, space="PSUM") as psum_pool,
        ):
            a5_t = desc_pool.tile([D, G, 128], f8e4, tag="a5")
            bm_t = desc_pool.tile([D, IJ], f8e4, tag="bm")
            id_t = desc_pool.tile([D, D], f8e5, tag="idn")
            bias_a = desc_pool.tile([128, 1], f32, tag="ba")
            bias_b = desc_pool.tile([128, 1], f32, tag="bb")
            # split the big input DMAs so the first matmuls can start early:
            # the first half-group needs only a5[g0], the first 2048 bm
            # columns, the identity and its own mask
            nc.sync.dma_start(a5_t[:, 0, :], a5[:, 0, :])
            nc.sync.dma_start(bm_t[:, :HG_COLS], bm[:, :HG_COLS])
            nc.sync.dma_start(id_t[:], idn[:])
            nc.gpsimd.memset(bias_a[:], -1.0)
            nc.gpsimd.memset(bias_b[:], -THR2)

            accA_t = acc_pool.tile([128, N_HG], f32, tag="accsA")
            accB_t = acc_pool.tile([128, N_HG], f32, tag="accsB")

            # Warm up the Tensor engine during the initial DMA wait: HAM
            # un-throttles after ~3.4us of continuous PE activity, so dummy
            # matmuls on memset data (no DMA dependency) make the real
            # matmuls start at full clock. The dummy activation also hoists
            # the one-time ACT_TABLE_LOAD off the critical path.
            dm_t = desc_pool.tile([128, 512], f8e4, tag="warm")
            dwo = desc_pool.tile([128, 1], f32, tag="warmout")
            nc.gpsimd.memset(dm_t[:], 1.0)
            psum_w = psum_pool.tile([128, HG_COLS], f32, tag="d")
            for _ in range(6):
                nc.tensor.matmul(
                    psum_w[:, :512], dm_t[:, :128], dm_t[:],
                    start=True, stop=True, skip_group_check=True,
                )
            nc.scalar.activation(
                dwo[:], psum_w[:, :1], Act.Abs, bias=bias_a[:], scale=1.0,
            )

            for hg in range(N_HG):
                g, h = hg // 2, hg % 2
                c0 = h * HG_COLS

                mm_t = mask_pool.tile([128, HG_COLS], f8e5, tag="m8")
                nc.sync.dma_start(mm_t[:], m8[:, hg, :])
                if hg == 0:
                    # bulk loads issued after the first mask so hg0's inject
                    # chain is not stuck behind them in the DMA issue queue
                    nc.sync.dma_start(bm_t[:, HG_COLS:], bm[:, HG_COLS:])
                    nc.sync.dma_start(a5_t[:, 1:, :], a5[:, 1:, :])

                psum_t = psum_pool.tile([128, HG_COLS], f32, tag="d")
                for j in range(HG_COLS // MMF):
                    js = slice(j * MMF, (j + 1) * MMF)
                    cs = slice(c0 + j * MMF, c0 + (j + 1) * MMF)
                    nc.tensor.matmul(
                        psum_t[:, js], a5_t[:, g, :], bm_t[:, cs],
                        start=True, stop=False,
                    )
                for j in range(HG_COLS // MMF):
                    js = slice(j * MMF, (j + 1) * MMF)
                    nc.tensor.matmul(
                        psum_t[:, js], id_t[:], mm_t[:, js],
                        start=False, stop=True,
                    )

                out1 = o1_pool.tile([128, HG_COLS], f16, tag="o1")
                nc.scalar.activation(
                    out1[:], psum_t[:], Act.Abs,
                    bias=bias_a[:], scale=1.0,
                    accum_out=accA_t[:, hg:hg + 1],
                )
                g2 = g_pool.tile([128, HG_COLS], f16, tag="g")
                if hg == N_HG - 1:
                    # last hg's B-pass on ACT: ACT's stream ends one pass
                    # before DVE's, so this trims the DVE tail
                    nc.scalar.activation(
                        g2[:], out1[:], Act.Relu,
                        bias=bias_b[:], scale=1.0,
                        accum_out=accB_t[:, hg:hg + 1],
                    )
                else:
                    nc.vector.tensor_scalar(
                        g2[:], out1[:], THR2, None,
                        op0=Alu.max, op1=Alu.add,
                        accum_out=accB_t[:, hg:hg + 1],
                    )

            nc.sync.dma_start(accs_out[:, :N_HG], accA_t[:])
            nc.sync.dma_start(accs_out[:, N_HG:], accB_t[:])

    nc.finalize()
    return nc


def _prep_inputs(descriptors_0, descriptors_1, similarity_mask):
    d0 = np.asarray(descriptors_0, dtype=np.float32)
    d1 = np.asarray(descriptors_1, dtype=np.float32)
    mkv = np.asarray(similarity_mask)
    idn = np.zeros((D, D), dtype=np.float32)
    np.fill_diagonal(idn, -1.0)
    idn = np.ascontiguousarray(idn.astype(ml_dtypes.float8_e5m2))
    in_maps = []
    su_list = []
    for c in range(N_CORES):
        b = c >> 2
        r0 = (c & 3) * ROWS_PER_CORE
        a5 = (d0[b].reshape(D, IJ)[:, r0:r0 + ROWS_PER_CORE] * np.float32(5.0))
        a5q8 = a5.astype(ml_dtypes.float8_e4m3fn)
        a5q = a5q8.astype(np.float32)            # [128 chan, 1024 rows]
        bmq8 = d1[b].reshape(D, IJ).astype(ml_dtypes.float8_e4m3fn)
        bmq = bmq8.astype(np.float32)
        mk = mkv[b].reshape(IJ, IJ)[r0:r0 + ROWS_PER_CORE]  # [1024, 4096] bool
        # [row(G,128), col(2,2048)] -> [128, hg=(G,2), 2048]
        m4k = (mk.astype(np.float32) * np.float32(C)).reshape(G, 128, 2, HG_COLS)
        m8v = np.ascontiguousarray(m4k.transpose(1, 0, 2, 3).reshape(
            128, N_HG, HG_COLS
        )).astype(ml_dtypes.float8_e5m2)
        # host-exact Su_total = sum(u) over the whole slab
        bsum = bmq.sum(axis=1, dtype=np.float64)
        su_tot = float((a5q.astype(np.float64).T @ bsum).sum()) - float(C) * float(
            mk.sum(dtype=np.int64)
        )
        su_list.append(su_tot)
        in_maps.append(
            {
                "a5": np.ascontiguousarray(a5q8.reshape(D, G, 128)),
                "bm": np.ascontiguousarray(bmq8),
                "idn": idn,
                "m8": m8v,
            }
        )
    _cached["su"] = su_list
    return in_maps


def _run(in_maps, **kwargs):
    if "nc" not in _cached:
        _cached["nc"] = _build_program()
    return run_bass_kernel_spmd(_cached["nc"], in_maps, list(range(N_CORES)), **kwargs)


def _combine(results):
    su_list = _cached["su"]
    n_core = ROWS_PER_CORE * IJ                  # elements per core
    total = 0.0
    for c, r in enumerate(results):
        accs = r["accs"].astype(np.float64)      # [128, 2*N_HG]
        acc1 = accs[:, :N_HG].sum()
        acc2 = accs[:, N_HG:].sum()
        A = 0.5 * (su_list[c] - n_core) + 0.5 * acc1
        # last hg's B-accum is relu(out1-2044) = B directly; the other 15
        # accumulate max(out1, 2044) = B + 2044*N
        Bv = acc2 - THR2 * (n_core - 128 * HG_COLS)
        total += (A + 250.0 * Bv) / 5.0
    return np.float32(total / float(B * IJ * IJ))


def kernel(descriptors_0, descriptors_1, similarity_mask):
    in_maps = _prep_inputs(descriptors_0, descriptors_1, similarity_mask)
    res = _run(in_maps)
    return _combine(res.results)
